# revision 19
# baseline (speedup 1.0000x reference)
"""Bayesian NN Monte-Carlo sampling kernel for 8 TRN2 NeuronCores.

Shards the n_samples axis (S=100 -> 13 per core, 4 padded) across 8 cores.
All math is general (std computed on device from the logvar tensors); host
prep is layout/dtype-only (bf16 cast + reshape/transpose/zero-pad).

Layout: features interleaved mod 4, contraction rows grouped p-major, and
the eps streams stored TRANSPOSED on the host so the grouped loads ride the
DMA-transpose XBAR path (higher effective GB/s than the plain-descriptor
path, which is capped ~17 GB/s per SDMA engine write-side). Each layer's
relu output lands exactly in the next layer's contraction layout (partition
p holds features 4p..4p+3) -> no transposes in the compute path.

Engine split (all matmuls bf16):
  DVE: in-place per-sample dequant muls (2x perf mode), t0-1 half of the
       layer-1 mean fold; GPSIMD adds the t2-3 half.
  PE:  psum[128,256] per layer; layer-0 psum initialized with precomputed
       y0T = x@wm0 via an identity matmul (DVE never waits on PSUM).
  ACT: per-chunk biased relus straight from psum; one output copy at end.
"""

import os
import sys

import numpy as np

if "/opt/trn_rl_repo" not in sys.path:
    sys.path.insert(0, "/opt/trn_rl_repo")

import concourse.bass as bass
from concourse import bacc, mybir, tile
from concourse.bass_utils import run_bass_kernel_spmd

S, B = 100, 64
D0, D1, D2, DO = 784, 512, 512, 10
NCORES = 8
SP = 13           # samples per core; 8*13 = 104, last 4 are wrap padding
P0, T0 = 112, 7   # layer-0 contraction: k = 7*p + t (p-major)
P1, T1 = 128, 4   # layer-1/2 contraction: k = 4*p + t (p-major)
C1 = 4            # feature chunks (features 4*q + c on chunk c, partition q)
W0C, W1C = T0 * D1, T1 * D2   # per-sample eps columns: 3584, 2048
GROUPS = [(0, 1), (1, 3), (3, 6), (6, 9), (9, 13)]
GMAX = 4
H1 = W1C // 2

F32 = mybir.dt.float32
BF16 = mybir.dt.bfloat16

_CACHE = {}


def _build(mode="bf16"):
    io_dt = BF16
    ts = bass.ts
    AF = mybir.ActivationFunctionType

    nc = bacc.Bacc("TRN2", target_bir_lowering=False, debug=False,
                   num_devices=NCORES)

    def inp(name, shape, dt=io_dt):
        return nc.dram_tensor(name, shape, dt, kind="ExternalInput").ap()

    # p-major / mod-4-interleaved host layouts (see _prep_in_maps)
    xT = inp("xT", [P0, T0 * B])
    wm0 = inp("wm0", [P0, W0C])
    wv0 = inp("wv0", [P0, W0C])
    wm1 = inp("wm1", [P1, W1C])
    wv1 = inp("wv1", [P1, W1C])
    wmlT = inp("wmlT", [P1, T1 * DO])
    wvlT = inp("wvlT", [P1, T1 * DO])
    welT = inp("welT", [P1, SP * T1 * DO])
    we0A = inp("we0A", [P0, SP * W0C])
    we1A = inp("we1A", [P1, SP * W1C], mybir.dt.int8)
    eye128 = inp("eye128", [P1, P1])

    bR = inp("bR", [SP, 6 * D1])   # [bv0r|bm0r|be0p|bv1r|bm1r|be1p] bf16
    bvl = inp("bvl", [SP, DO])
    bml = inp("bml", [SP, DO])
    bel = inp("bel", [SP, DO])
    ind = inp("ind", [SP, SP * B])
    out = nc.dram_tensor("out", [B, SP * DO], F32, kind="ExternalOutput").ap()

    with tile.TileContext(nc) as tc:
        with tc.tile_pool(name="const", bufs=1) as const, \
             tc.tile_pool(name="w0g", bufs=4) as w0g, \
             tc.tile_pool(name="w1g", bufs=4) as w1g, \
             tc.tile_pool(name="w1c", bufs=3) as w1c, \
             tc.tile_pool(name="wls", bufs=2) as wls, \
             tc.tile_pool(name="acts", bufs=3) as acts, \
             tc.tile_pool(name="bias", bufs=1) as bias, \
             tc.tile_pool(name="ps", bufs=2, space="PSUM") as ps:

            # ---------------- one-time setup ----------------
            # scalar ring: wv0 first (gates the sample-0 dequant), then x/eye
            tmp0 = const.tile([P0, W0C], io_dt, tag="tmp0")
            nc.scalar.dma_start(tmp0[:], wv0[:, :])
            t_std0 = const.tile([P0, W0C], io_dt)
            nc.scalar.activation(t_std0[:], tmp0[:], AF.Exp, scale=0.5)

            t_xT = const.tile([P0, T0 * B], io_dt)
            nc.scalar.dma_start(t_xT[:], xT[:, :])
            t_eye = const.tile([P1, P1], io_dt)
            nc.scalar.dma_start(t_eye[:], eye128[:, :])

            tmp1 = const.tile([P1, W1C], io_dt, tag="tmp1")
            nc.scalar.dma_start(tmp1[:], wv1[:, :])
            t_std1 = const.tile([P1, W1C], io_dt)
            nc.scalar.activation(t_std1[:], tmp1[:], AF.Exp, scale=0.5)

            # scalar ring: small bias/last-layer tensors in parallel
            tmpl = wls.tile([P1, T1 * DO], io_dt, tag="t_wls")
            nc.scalar.dma_start(tmpl[:], wvlT[:, :])
            t_stdl = const.tile([P1, T1 * DO], io_dt)
            nc.scalar.activation(t_stdl[:], tmpl[:], AF.Exp, scale=0.5)
            t_wml = const.tile([P1, T1 * DO], io_dt)
            nc.scalar.dma_start(t_wml[:], wmlT[:, :])
            t_wel = const.tile([P1, SP * T1 * DO], io_dt)
            nc.scalar.dma_start(t_wel[:], welT[:, :])

            # hidden bias rows: bR0/bR1 [SP, 512] in permuted (c*128+q) order
            t_bR = bias.tile([SP, 6 * D1], io_dt, tag="bR")
            nc.scalar.dma_start(t_bR[:], bR[:, :])

            def make_bias_R(off, name):
                st = bias.tile([SP, D1], io_dt, tag=name + "s")
                nc.scalar.activation(st[:], t_bR[:, off: off + D1],
                                     AF.Exp, scale=0.5)
                bt = const.tile([SP, D1], io_dt, tag=name)
                nc.vector.tensor_mul(bt[:], t_bR[:, off + 2 * D1: off + 3 * D1],
                                     st[:])
                nc.vector.tensor_add(bt[:], bt[:],
                                     t_bR[:, off + D1: off + 2 * D1])
                return bt

            t_bR0 = make_bias_R(0, "bR0")
            t_bR1 = make_bias_R(3 * D1, "bR1")

            # last-layer bias rows [SP, DO]: bvl/bml pre-replicated on host
            r = bias.tile([SP, DO], io_dt, tag="brow")
            nc.scalar.dma_start(r[:], bvl[:, :])
            sbb = bias.tile([SP, DO], io_dt, tag="brow2")
            nc.scalar.activation(sbb[:], r[:], AF.Exp, scale=0.5)
            mb = bias.tile([SP, DO], io_dt, tag="brow3")
            nc.scalar.dma_start(mb[:], bml[:, :])
            eb = bias.tile([SP, DO], io_dt, tag="bb3")
            nc.scalar.dma_start(eb[:], bel[:, :])
            ba = bias.tile([SP, DO], io_dt, tag="bb4")
            nc.vector.tensor_mul(ba[:], eb[:], sbb[:])
            t_bl = bias.tile([SP, DO], io_dt, tag="ball")
            nc.vector.tensor_add(t_bl[:], ba[:], mb[:])

            t_ind = const.tile([SP, SP * B], io_dt)
            nc.scalar.dma_start(t_ind[:], ind[:, :])

            t_wm0 = const.tile([P0, W0C], io_dt)
            t_wm1 = const.tile([P1, W1C], io_dt)

            t_out = const.tile([B, SP * DO], F32)

            def mm(psum, lhsT, rhs, start, stop, skip=False):
                nc.tensor.matmul(psum, lhsT, rhs, start=start, stop=stop,
                                 skip_group_check=skip)

            # y0T[q, c*64+b] = (x @ wm0)[4q+c, b], precomputed once (bf16)
            def make_y0T():
                y0 = const.tile([P1, C1 * B], io_dt)
                py = ps.tile([P1, C1 * B], F32, tag="p0")
                for c in range(C1):
                    for t in range(T0):
                        mm(py[:, ts(c, B)],
                           t_wm0[:, t * D1 + c * P1: t * D1 + (c + 1) * P1],
                           t_xT[:, ts(t, B)],
                           start=(t == 0), stop=(t == T0 - 1))
                nc.scalar.copy(y0[:], py[:])
                return y0

            # ---------------- per-sample weight prep (conveyor) ----------------
            def weight_prep(s, first=False):
                t_e0 = w0g.tile([P0, W0C], io_dt, tag="t_e0")
                nc.sync.dma_start(t_e0[:], we0A[:, s * W0C: (s + 1) * W0C])
                if first:
                    nc.sync.dma_start(t_wm0[:], wm0[:, :])
                t_i8 = w1g.tile([P1, W1C], mybir.dt.int8, tag="t_i8")
                nc.sync.dma_start(t_i8[:], we1A[:, s * W1C: (s + 1) * W1C])
                if first:
                    nc.sync.dma_start(t_wm1[:], wm1[:, :])

                t_e1 = w1c.tile([P1, W1C], io_dt, tag="t_e1")
                nc.scalar.mul(t_e1[:], t_i8[:], 0.03125)
                nc.vector.tensor_mul(t_e0[:], t_e0[:], t_std0[:])
                nc.vector.tensor_mul(t_e1[:], t_e1[:], t_std1[:])
                nc.vector.tensor_add(t_e1[:], t_e1[:], t_wm1[:])
                t_wl = wls.tile([P1, T1 * DO], io_dt, tag="t_wlf")
                nc.vector.tensor_mul(
                    t_wl[:], t_wel[:, s * T1 * DO: (s + 1) * T1 * DO],
                    t_stdl[:])
                nc.vector.tensor_add(t_wl[:], t_wl[:], t_wml[:])
                return t_e0, t_e1, t_wl

            def compute(s, t_e0, t_e1, t_wl, t_y0T, po):
                w0 = t_e0[:]
                w1 = t_e1[:]
                wlf = t_wl[:]

                # layer 0: bias injected via K=1 matmul; single relu
                a1T = acts.tile([P1, C1 * B], io_dt, tag="a1T")
                p0 = ps.tile([P1, C1 * B], F32, tag="p0")
                for c in range(C1):
                    mm(p0[:, ts(c, B)], t_eye[:], t_y0T[:, ts(c, B)],
                       start=True, stop=False)
                    for t in range(T0):
                        mm(p0[:, ts(c, B)],
                           w0[:, t * D1 + c * P1: t * D1 + (c + 1) * P1],
                           t_xT[:, ts(t, B)],
                           start=False, stop=False)
                    mm(p0[:, ts(c, B)],
                       t_bR0[:, c * P1: (c + 1) * P1], t_ind[:, ts(s, B)],
                       start=False, stop=True)
                nc.scalar.activation(a1T[:], p0[:], AF.Relu)

                # layer 1 (mean already folded into w1)
                a2T = acts.tile([P1, C1 * B], io_dt, tag="a2T")
                p1 = ps.tile([P1, C1 * B], F32, tag="p1")
                for c in range(C1):
                    for t in range(T1):
                        mm(p1[:, ts(c, B)],
                           w1[:, t * D2 + c * P1: t * D2 + (c + 1) * P1],
                           a1T[:, ts(t, B)],
                           start=(t == 0), stop=False)
                    mm(p1[:, ts(c, B)],
                       t_bR1[:, c * P1: (c + 1) * P1], t_ind[:, ts(s, B)],
                       start=False, stop=True)
                nc.scalar.activation(a2T[:], p1[:], AF.Relu)

                # output layer: all samples share one [64, SP*DO] psum bank
                for t in range(T1):
                    mm(po[:, ts(s, DO)], a2T[:, ts(t, B)],
                       wlf[:, ts(t, DO)], start=(t == 0), stop=False)
                mm(po[:, ts(s, DO)], t_ind[:, ts(s, B)], t_bl[:],
                   start=False, stop=True)

            po = ps.tile([B, SP * DO], F32, tag="out")
            LOOKAHEAD = 3
            preps = [weight_prep(0, first=True)]
            t_y0T = make_y0T()
            for s in range(1, LOOKAHEAD):
                preps.append(weight_prep(s))
            for s in range(SP):
                compute(s, *preps[s], t_y0T, po)
                if s + LOOKAHEAD < SP:
                    preps.append(weight_prep(s + LOOKAHEAD))
            nc.scalar.copy(t_out[:], po[:])
            nc.sync.dma_start(out[:, :], t_out[:])

    nc.compile()
    return nc


def _get_nc(mode="bf16"):
    if "nc" not in _CACHE:
        _CACHE["nc"] = _build()
    return _CACHE["nc"]


def _prep_in_maps(inputs, mode="bf16"):
    import ml_dtypes
    np_dt = ml_dtypes.bfloat16

    def cvt(a):
        return np.ascontiguousarray(a).astype(np_dt, copy=False)

    x = np.asarray(inputs["inputs"], np.float32)
    we0 = np.asarray(inputs["we0"], np.float32)
    we1 = np.asarray(inputs["we1"], np.float32)
    wel = np.asarray(inputs["wel"], np.float32)
    be0 = np.asarray(inputs["be0"], np.float32).reshape(S, D1)
    be1 = np.asarray(inputs["be1"], np.float32).reshape(S, D2)
    bel = np.asarray(inputs["bel"], np.float32).reshape(S, DO)

    # p-major rows + mod-4 interleaved feature columns:
    #   out[p, (t, c, q)] = M[T*p + t, 4*q + c]
    def pm0(M):  # [784, 512] -> [112, 7*512]
        return M.reshape(P0, T0, P1, C1).transpose(0, 1, 3, 2) \
                .reshape(P0, W0C)

    def pm1(M):  # [512, 512] -> [128, 4*512]
        return M.reshape(P1, T1, P1, C1).transpose(0, 1, 3, 2) \
                .reshape(P1, W1C)

    def pml(M):  # [512, 10] -> [128, 4*10] (row permutation only)
        return M.reshape(P1, T1 * DO)

    xTpm = x.T.reshape(P0, T0, B).reshape(P0, T0 * B)

    def bias_T(b):  # [SP, D] -> [128, C1*SP] with [q, c*SP+s] = b[s, 4q+c]
        return np.ascontiguousarray(
            b.reshape(SP, P1, C1).transpose(1, 2, 0).reshape(P1, C1 * SP))

    def permf(v):  # feature vector [512] -> permuted [c*128+q] = v[4q+c]
        return np.ascontiguousarray(
            np.asarray(v, np.float32).reshape(P1, C1).T.reshape(-1))

    def q8(a):
        return np.clip(np.rint(np.ascontiguousarray(a) * 32.0),
                       -127, 127).astype(np.int8)

    shared = {
        "xT": cvt(xTpm),
        "wm0": cvt(pm0(np.asarray(inputs["wm0"], np.float32))),
        "wv0": cvt(pm0(np.asarray(inputs["wv0"], np.float32))),
        "wm1": cvt(pm1(np.asarray(inputs["wm1"], np.float32))),
        "wv1": cvt(pm1(np.asarray(inputs["wv1"], np.float32))),
        "wmlT": cvt(pml(np.asarray(inputs["wml"], np.float32))),
        "wvlT": cvt(pml(np.asarray(inputs["wvl"], np.float32))),
        "eye128": cvt(np.eye(P1, dtype=np.float32)),
        "bvl": cvt(np.repeat(np.asarray(inputs["bvl"], np.float32)
                             .reshape(1, DO), SP, axis=0)),
        "bml": cvt(np.repeat(np.asarray(inputs["bml"], np.float32)
                             .reshape(1, DO), SP, axis=0)),
        "ind": cvt(np.repeat(np.eye(SP, dtype=np.float32), B, axis=1)),
    }

    def shard(a, k):
        lo = k * SP
        hi = lo + SP
        if hi <= S:
            return a[lo:hi]
        return np.concatenate([a[lo:S], a[: hi - S]], axis=0)

    def q8(a):
        return np.clip(np.rint(np.ascontiguousarray(a) * 32.0),
                       -127, 127).astype(np.int8)

    in_maps = []
    for k in range(NCORES):
        welk = shard(wel, k)  # [SP, 512, 10]
        be0p = shard(be0, k)[:, :].reshape(SP, P1, C1) \
            .transpose(0, 2, 1).reshape(SP, D1)
        be1p = shard(be1, k)[:, :].reshape(SP, P1, C1) \
            .transpose(0, 2, 1).reshape(SP, D2)
        bRk = np.concatenate([
            np.repeat(permf(inputs["bv0"])[None], SP, axis=0),
            np.repeat(permf(inputs["bm0"])[None], SP, axis=0),
            be0p,
            np.repeat(permf(inputs["bv1"])[None], SP, axis=0),
            np.repeat(permf(inputs["bm1"])[None], SP, axis=0),
            be1p,
        ], axis=1)
        in_maps.append(dict(
            shared,
            we0A=cvt(np.stack([pm0(m) for m in shard(we0, k)], axis=1)
                     .reshape(P0, SP * W0C)),
            we1A=q8(np.stack([pm1(m) for m in shard(we1, k)], axis=1)
                    .reshape(P1, SP * W1C)),
            welT=cvt(np.stack([pml(m) for m in welk], axis=1)
                     .reshape(P1, SP * T1 * DO)),
            bR=cvt(bRk),
            bel=cvt(shard(bel, k)),
        ))
    return in_maps


def _run(inputs, mode="bf16", trace=False):
    nc = _get_nc(mode)
    in_maps = _prep_in_maps(inputs, mode)
    res = run_bass_kernel_spmd(nc, in_maps, core_ids=list(range(NCORES)),
                               trace=trace)
    outs = []
    for k in range(NCORES):
        o = np.asarray(res.results[k]["out"], np.float32)  # [64, 130]
        outs.append(o.reshape(B, SP, DO).transpose(1, 0, 2))
    full = np.concatenate(outs, axis=0)[:S]  # [100, 64, 10]
    return full, res


def kernel(**inputs):
    out, _ = _run(inputs)
    return out


# revision 20
# speedup vs baseline: 1.2000x; 1.2000x over previous
"""Bayesian NN Monte-Carlo sampling kernel for 8 TRN2 NeuronCores.

Shards the n_samples axis (S=100 -> 13 per core, 4 padded) across 8 cores.
All math is general (std computed on device from the logvar tensors); host
prep is layout/dtype-only (bf16 cast + reshape/transpose/zero-pad).

Layout: features interleaved mod 4, contraction rows grouped p-major, and
the eps streams stored TRANSPOSED on the host so the grouped loads ride the
DMA-transpose XBAR path (higher effective GB/s than the plain-descriptor
path, which is capped ~17 GB/s per SDMA engine write-side). Each layer's
relu output lands exactly in the next layer's contraction layout (partition
p holds features 4p..4p+3) -> no transposes in the compute path.

Engine split (all matmuls bf16):
  DVE: in-place per-sample dequant muls (2x perf mode), t0-1 half of the
       layer-1 mean fold; GPSIMD adds the t2-3 half.
  PE:  psum[128,256] per layer; layer-0 psum initialized with precomputed
       y0T = x@wm0 via an identity matmul (DVE never waits on PSUM).
  ACT: per-chunk biased relus straight from psum; one output copy at end.
"""

import os
import sys

import numpy as np

if "/opt/trn_rl_repo" not in sys.path:
    sys.path.insert(0, "/opt/trn_rl_repo")

import concourse.bass as bass
from concourse import bacc, mybir, tile
from concourse.bass_utils import run_bass_kernel_spmd

S, B = 100, 64
D0, D1, D2, DO = 784, 512, 512, 10
NCORES = 8
SP = 13           # samples per core; 8*13 = 104, last 4 are wrap padding
P0, T0 = 112, 7   # layer-0 contraction: k = 7*p + t (p-major)
P1, T1 = 128, 4   # layer-1/2 contraction: k = 4*p + t (p-major)
C1 = 4            # feature chunks (features 4*q + c on chunk c, partition q)
W0C, W1C = T0 * D1, T1 * D2   # per-sample eps columns: 3584, 2048
GROUPS = [(0, 1), (1, 3), (3, 6), (6, 9), (9, 13)]
GMAX = 4
H1 = W1C // 2

F32 = mybir.dt.float32
BF16 = mybir.dt.bfloat16

_CACHE = {}


def _build(mode="bf16"):
    io_dt = BF16
    ts = bass.ts
    AF = mybir.ActivationFunctionType

    nc = bacc.Bacc("TRN2", target_bir_lowering=False, debug=False,
                   num_devices=NCORES)

    def inp(name, shape, dt=io_dt):
        return nc.dram_tensor(name, shape, dt, kind="ExternalInput").ap()

    # p-major / mod-4-interleaved host layouts (see _prep_in_maps)
    xT = inp("xT", [P0, T0 * B])
    wm0 = inp("wm0", [P0, W0C])
    wv0 = inp("wv0", [P0, W0C])
    wm1 = inp("wm1", [P1, W1C])
    wv1 = inp("wv1", [P1, W1C])
    wmlT = inp("wmlT", [P1, T1 * DO])
    wvlT = inp("wvlT", [P1, T1 * DO])
    welT = inp("welT", [P1, SP * T1 * DO])
    we0A = inp("we0A", [P0, SP * W0C])
    we1A = inp("we1A", [P1, SP * W1C])
    eye128 = inp("eye128", [P1, P1])

    b01 = inp("b01", [P1, 2 * (2 * C1 + C1 * SP)], F32)  # packed hidden biases
    bvl = inp("bvl", [SP, DO])
    bml = inp("bml", [SP, DO])
    bel = inp("bel", [SP, DO])
    ind = inp("ind", [SP, SP * B])
    out = nc.dram_tensor("out", [B, SP * DO], F32, kind="ExternalOutput").ap()

    with tile.TileContext(nc) as tc:
        with tc.tile_pool(name="const", bufs=1) as const, \
             tc.tile_pool(name="w0g", bufs=5) as w0g, \
             tc.tile_pool(name="w1g", bufs=5) as w1g, \
             tc.tile_pool(name="wls", bufs=2) as wls, \
             tc.tile_pool(name="acts", bufs=3) as acts, \
             tc.tile_pool(name="bias", bufs=1) as bias, \
             tc.tile_pool(name="ps", bufs=1, space="PSUM") as ps:

            # ---------------- one-time setup ----------------
            # scalar ring: wv0 first (gates the sample-0 dequant), then x/eye
            tmp0 = const.tile([P0, W0C], io_dt, tag="tmp0")
            nc.scalar.dma_start(tmp0[:], wv0[:, :])
            t_std0 = const.tile([P0, W0C], io_dt)
            nc.scalar.activation(t_std0[:], tmp0[:], AF.Exp, scale=0.5)

            t_xT = const.tile([P0, T0 * B], io_dt)
            nc.scalar.dma_start(t_xT[:], xT[:, :])
            t_eye = const.tile([P1, P1], io_dt)
            nc.scalar.dma_start(t_eye[:], eye128[:, :])

            tmp1 = const.tile([P1, W1C], io_dt, tag="tmp1")
            nc.scalar.dma_start(tmp1[:], wv1[:, :])
            t_std1 = const.tile([P1, W1C], io_dt)
            nc.scalar.activation(t_std1[:], tmp1[:], AF.Exp, scale=0.5)

            # scalar ring: small bias/last-layer tensors in parallel
            tmpl = wls.tile([P1, T1 * DO], io_dt, tag="t_wls")
            nc.scalar.dma_start(tmpl[:], wvlT[:, :])
            t_stdl = const.tile([P1, T1 * DO], io_dt)
            nc.scalar.activation(t_stdl[:], tmpl[:], AF.Exp, scale=0.5)
            t_wml = const.tile([P1, T1 * DO], io_dt)
            nc.scalar.dma_start(t_wml[:], wmlT[:, :])
            t_wel = const.tile([P1, SP * T1 * DO], io_dt)
            nc.scalar.dma_start(t_wel[:], welT[:, :])

            # packed hidden biases: [bv0|bm0|be0|bv1|bm1|be1] along free dim
            CB = 2 * C1 + C1 * SP
            t_b01 = bias.tile([P1, 2 * CB], F32, tag="b01")
            nc.scalar.dma_start(t_b01[:], b01[:, :])

            def make_bias_T(off, name):
                vt = t_b01[:, off: off + C1]
                mt = t_b01[:, off + C1: off + 2 * C1]
                et = t_b01[:, off + 2 * C1: off + CB]
                st = bias.tile([P1, C1], F32, tag=name + "s")
                nc.scalar.activation(st[:], vt, AF.Exp, scale=0.5)
                bt = const.tile([P1, C1 * SP], F32, tag=name)
                for c in range(C1):
                    nc.vector.tensor_scalar_mul(
                        bt[:, ts(c, SP)], et[:, ts(c, SP)], st[:, c:c + 1])
                    nc.vector.tensor_scalar_add(
                        bt[:, ts(c, SP)], bt[:, ts(c, SP)], mt[:, c:c + 1])
                return bt

            t_bT0 = make_bias_T(0, "bT0")
            t_bT1 = make_bias_T(CB, "bT1")

            # last-layer bias rows [SP, DO]: bvl/bml pre-replicated on host
            r = bias.tile([SP, DO], io_dt, tag="brow")
            nc.scalar.dma_start(r[:], bvl[:, :])
            sbb = bias.tile([SP, DO], io_dt, tag="brow2")
            nc.scalar.activation(sbb[:], r[:], AF.Exp, scale=0.5)
            mb = bias.tile([SP, DO], io_dt, tag="brow3")
            nc.scalar.dma_start(mb[:], bml[:, :])
            eb = bias.tile([SP, DO], io_dt, tag="bb3")
            nc.scalar.dma_start(eb[:], bel[:, :])
            ba = bias.tile([SP, DO], io_dt, tag="bb4")
            nc.vector.tensor_mul(ba[:], eb[:], sbb[:])
            t_bl = bias.tile([SP, DO], io_dt, tag="ball")
            nc.vector.tensor_add(t_bl[:], ba[:], mb[:])

            t_ind = const.tile([SP, SP * B], io_dt)
            nc.scalar.dma_start(t_ind[:], ind[:, :])

            t_wm0 = const.tile([P0, W0C], io_dt)
            t_wm1 = const.tile([P1, W1C], io_dt)

            t_out = const.tile([B, SP * DO], F32)

            def mm(psum, lhsT, rhs, start, stop, skip=False):
                nc.tensor.matmul(psum, lhsT, rhs, start=start, stop=stop,
                                 skip_group_check=skip)

            # y0T[q, c*64+b] = (x @ wm0)[4q+c, b], precomputed once (bf16)
            def make_y0T():
                y0 = const.tile([P1, C1 * B], io_dt)
                for c in range(C1):
                    pc = ps.tile([P1, B], F32, tag=f"pc{c}")
                    for t in range(T0):
                        mm(pc[:],
                           t_wm0[:, t * D1 + c * P1: t * D1 + (c + 1) * P1],
                           t_xT[:, ts(t, B)],
                           start=(t == 0), stop=(t == T0 - 1))
                    nc.scalar.copy(y0[:, ts(c, B)], pc[:])
                return y0

            # ---------------- per-sample weight prep (conveyor) ----------------
            def weight_prep(s, first=False):
                t_e0 = w0g.tile([P0, W0C], io_dt, tag="t_e0")
                nc.sync.dma_start(t_e0[:], we0A[:, s * W0C: (s + 1) * W0C])
                if first:
                    nc.sync.dma_start(t_wm0[:], wm0[:, :])
                t_e1 = w1g.tile([P1, W1C], io_dt, tag="t_e1")
                nc.sync.dma_start(t_e1[:], we1A[:, s * W1C: (s + 1) * W1C])
                if first:
                    nc.sync.dma_start(t_wm1[:], wm1[:, :])

                nc.vector.tensor_mul(t_e0[:], t_e0[:], t_std0[:])
                nc.vector.tensor_mul(t_e1[:], t_e1[:], t_std1[:])
                nc.vector.tensor_add(t_e1[:], t_e1[:], t_wm1[:])
                t_wl = wls.tile([P1, T1 * DO], io_dt, tag="t_wlf")
                nc.vector.tensor_mul(
                    t_wl[:], t_wel[:, s * T1 * DO: (s + 1) * T1 * DO],
                    t_stdl[:])
                nc.vector.tensor_add(t_wl[:], t_wl[:], t_wml[:])
                return t_e0, t_e1, t_wl

            def compute(s, t_e0, t_e1, t_wl, t_y0T, po):
                w0 = t_e0[:]
                w1 = t_e1[:]
                wlf = t_wl[:]

                # layer 0: per-chunk psum tiles so relu(c) overlaps mm(c+1)
                a1T = acts.tile([P1, C1 * B], io_dt, tag="a1T")
                for c in range(C1):
                    pc = ps.tile([P1, B], F32, tag=f"pc{c}")
                    mm(pc[:], t_eye[:], t_y0T[:, ts(c, B)],
                       start=True, stop=False)
                    for t in range(T0):
                        mm(pc[:],
                           w0[:, t * D1 + c * P1: t * D1 + (c + 1) * P1],
                           t_xT[:, ts(t, B)],
                           start=False, stop=(t == T0 - 1))
                    nc.scalar.activation(
                        a1T[:, ts(c, B)], pc[:], AF.Relu,
                        bias=t_bT0[:, c * SP + s: c * SP + s + 1])

                # layer 1 (mean already folded into w1)
                a2T = acts.tile([P1, C1 * B], io_dt, tag="a2T")
                for c in range(C1):
                    qc = ps.tile([P1, B], F32, tag=f"pc{c}")
                    for t in range(T1):
                        mm(qc[:],
                           w1[:, t * D2 + c * P1: t * D2 + (c + 1) * P1],
                           a1T[:, ts(t, B)],
                           start=(t == 0), stop=(t == T1 - 1))
                    nc.scalar.activation(
                        a2T[:, ts(c, B)], qc[:], AF.Relu,
                        bias=t_bT1[:, c * SP + s: c * SP + s + 1])

                # output layer: all samples share one [64, SP*DO] psum bank
                for t in range(T1):
                    mm(po[:, ts(s, DO)], a2T[:, ts(t, B)],
                       wlf[:, ts(t, DO)], start=(t == 0), stop=False)
                mm(po[:, ts(s, DO)], t_ind[:, ts(s, B)], t_bl[:],
                   start=False, stop=True)

            po = ps.tile([B, SP * DO], F32, tag="out")
            LOOKAHEAD = 4
            preps = [weight_prep(0, first=True)]
            t_y0T = make_y0T()
            for s in range(1, LOOKAHEAD):
                preps.append(weight_prep(s))
            for s in range(SP):
                compute(s, *preps[s], t_y0T, po)
                if s + LOOKAHEAD < SP:
                    preps.append(weight_prep(s + LOOKAHEAD))
            nc.scalar.copy(t_out[:], po[:])
            nc.sync.dma_start(out[:, :], t_out[:])

    nc.compile()
    return nc


def _get_nc(mode="bf16"):
    if "nc" not in _CACHE:
        _CACHE["nc"] = _build()
    return _CACHE["nc"]


def _prep_in_maps(inputs, mode="bf16"):
    import ml_dtypes
    np_dt = ml_dtypes.bfloat16

    def cvt(a):
        return np.ascontiguousarray(a).astype(np_dt, copy=False)

    x = np.asarray(inputs["inputs"], np.float32)
    we0 = np.asarray(inputs["we0"], np.float32)
    we1 = np.asarray(inputs["we1"], np.float32)
    wel = np.asarray(inputs["wel"], np.float32)
    be0 = np.asarray(inputs["be0"], np.float32).reshape(S, D1)
    be1 = np.asarray(inputs["be1"], np.float32).reshape(S, D2)
    bel = np.asarray(inputs["bel"], np.float32).reshape(S, DO)

    # p-major rows + mod-4 interleaved feature columns:
    #   out[p, (t, c, q)] = M[T*p + t, 4*q + c]
    def pm0(M):  # [784, 512] -> [112, 7*512]
        return M.reshape(P0, T0, P1, C1).transpose(0, 1, 3, 2) \
                .reshape(P0, W0C)

    def pm1(M):  # [512, 512] -> [128, 4*512]
        return M.reshape(P1, T1, P1, C1).transpose(0, 1, 3, 2) \
                .reshape(P1, W1C)

    def pml(M):  # [512, 10] -> [128, 4*10] (row permutation only)
        return M.reshape(P1, T1 * DO)

    xTpm = x.T.reshape(P0, T0, B).reshape(P0, T0 * B)

    def bias_T(b):  # [SP, D] -> [128, C1*SP] with [q, c*SP+s] = b[s, 4q+c]
        return np.ascontiguousarray(
            b.reshape(SP, P1, C1).transpose(1, 2, 0).reshape(P1, C1 * SP))

    def bias_cq(v):  # [D] -> [128, C1] with [q, c] = v[4q+c]
        return np.ascontiguousarray(np.asarray(v, np.float32)
                                    .reshape(P1, C1))

    shared = {
        "xT": cvt(xTpm),
        "wm0": cvt(pm0(np.asarray(inputs["wm0"], np.float32))),
        "wv0": cvt(pm0(np.asarray(inputs["wv0"], np.float32))),
        "wm1": cvt(pm1(np.asarray(inputs["wm1"], np.float32))),
        "wv1": cvt(pm1(np.asarray(inputs["wv1"], np.float32))),
        "wmlT": cvt(pml(np.asarray(inputs["wml"], np.float32))),
        "wvlT": cvt(pml(np.asarray(inputs["wvl"], np.float32))),
        "eye128": cvt(np.eye(P1, dtype=np.float32)),
        "bvl": cvt(np.repeat(np.asarray(inputs["bvl"], np.float32)
                             .reshape(1, DO), SP, axis=0)),
        "bml": cvt(np.repeat(np.asarray(inputs["bml"], np.float32)
                             .reshape(1, DO), SP, axis=0)),
        "ind": cvt(np.repeat(np.eye(SP, dtype=np.float32), B, axis=1)),
    }

    def shard(a, k):
        lo = k * SP
        hi = lo + SP
        if hi <= S:
            return a[lo:hi]
        return np.concatenate([a[lo:S], a[: hi - S]], axis=0)

    def q8(a):
        return np.clip(np.rint(np.ascontiguousarray(a) * 32.0),
                       -127, 127).astype(np.int8)

    in_maps = []
    for k in range(NCORES):
        welk = shard(wel, k)  # [SP, 512, 10]
        b0 = np.concatenate([bias_cq(inputs["bv0"]), bias_cq(inputs["bm0"]),
                             bias_T(shard(be0, k))], axis=1)
        b1 = np.concatenate([bias_cq(inputs["bv1"]), bias_cq(inputs["bm1"]),
                             bias_T(shard(be1, k))], axis=1)
        in_maps.append(dict(
            shared,
            we0A=cvt(np.stack([pm0(m) for m in shard(we0, k)], axis=1)
                     .reshape(P0, SP * W0C)),
            we1A=cvt(np.stack([pm1(m) for m in shard(we1, k)], axis=1)
                     .reshape(P1, SP * W1C)),
            welT=cvt(np.stack([pml(m) for m in welk], axis=1)
                     .reshape(P1, SP * T1 * DO)),
            b01=np.ascontiguousarray(np.concatenate([b0, b1], axis=1)),
            bel=cvt(shard(bel, k)),
        ))
    return in_maps


def _run(inputs, mode="bf16", trace=False):
    nc = _get_nc(mode)
    in_maps = _prep_in_maps(inputs, mode)
    res = run_bass_kernel_spmd(nc, in_maps, core_ids=list(range(NCORES)),
                               trace=trace)
    outs = []
    for k in range(NCORES):
        o = np.asarray(res.results[k]["out"], np.float32)  # [64, 130]
        outs.append(o.reshape(B, SP, DO).transpose(1, 0, 2))
    full = np.concatenate(outs, axis=0)[:S]  # [100, 64, 10]
    return full, res


def kernel(**inputs):
    out, _ = _run(inputs)
    return out


# revision 21
# speedup vs baseline: 1.2145x; 1.0121x over previous
"""Bayesian NN Monte-Carlo sampling kernel for 8 TRN2 NeuronCores.

Shards the n_samples axis (S=100 -> 13 per core, 4 padded) across 8 cores.
All math is general (std computed on device from the logvar tensors); host
prep is layout/dtype-only (bf16 cast + reshape/transpose/zero-pad).

Layout: features interleaved mod 4, contraction rows grouped p-major, and
the eps streams stored TRANSPOSED on the host so the grouped loads ride the
DMA-transpose XBAR path (higher effective GB/s than the plain-descriptor
path, which is capped ~17 GB/s per SDMA engine write-side). Each layer's
relu output lands exactly in the next layer's contraction layout (partition
p holds features 4p..4p+3) -> no transposes in the compute path.

Engine split (all matmuls bf16):
  DVE: in-place per-sample dequant muls (2x perf mode), t0-1 half of the
       layer-1 mean fold; GPSIMD adds the t2-3 half.
  PE:  psum[128,256] per layer; layer-0 psum initialized with precomputed
       y0T = x@wm0 via an identity matmul (DVE never waits on PSUM).
  ACT: per-chunk biased relus straight from psum; one output copy at end.
"""

import os
import sys

import numpy as np

if "/opt/trn_rl_repo" not in sys.path:
    sys.path.insert(0, "/opt/trn_rl_repo")

import concourse.bass as bass
from concourse import bacc, mybir, tile
from concourse.bass_utils import run_bass_kernel_spmd

S, B = 100, 64
D0, D1, D2, DO = 784, 512, 512, 10
NCORES = 8
SP = 13           # samples per core; 8*13 = 104, last 4 are wrap padding
P0, T0 = 112, 7   # layer-0 contraction: k = 7*p + t (p-major)
P1, T1 = 128, 4   # layer-1/2 contraction: k = 4*p + t (p-major)
C1 = 4            # feature chunks (features 4*q + c on chunk c, partition q)
W0C, W1C = T0 * D1, T1 * D2   # per-sample eps columns: 3584, 2048
GROUPS = [(0, 1), (1, 3), (3, 6), (6, 9), (9, 13)]
GMAX = 4
H1 = W1C // 2

F32 = mybir.dt.float32
BF16 = mybir.dt.bfloat16

_CACHE = {}


def _build(mode="bf16"):
    io_dt = BF16
    ts = bass.ts
    AF = mybir.ActivationFunctionType

    nc = bacc.Bacc("TRN2", target_bir_lowering=False, debug=False,
                   num_devices=NCORES)

    def inp(name, shape, dt=io_dt):
        return nc.dram_tensor(name, shape, dt, kind="ExternalInput").ap()

    # p-major / mod-4-interleaved host layouts (see _prep_in_maps)
    xT = inp("xT", [P0, T0 * B])
    wm0 = inp("wm0", [P0, W0C])
    wv0 = inp("wv0", [P0, W0C])
    wm1 = inp("wm1", [P1, W1C])
    wv1 = inp("wv1", [P1, W1C])
    wmlT = inp("wmlT", [P1, T1 * DO])
    wvlT = inp("wvlT", [P1, T1 * DO])
    welT = inp("welT", [P1, SP * T1 * DO])
    we0A = inp("we0A", [P0, SP * W0C])
    we1A = inp("we1A", [P1, SP * W1C])
    eye128 = inp("eye128", [P1, P1])

    b01 = inp("b01", [P1, 2 * (2 * C1 + C1 * SP)], F32)  # packed hidden biases
    bvl = inp("bvl", [SP, DO])
    bml = inp("bml", [SP, DO])
    bel = inp("bel", [SP, DO])
    ind = inp("ind", [SP, SP * B])
    out = nc.dram_tensor("out", [B, SP * DO], F32, kind="ExternalOutput").ap()

    with tile.TileContext(nc) as tc:
        with tc.tile_pool(name="const", bufs=1) as const, \
             tc.tile_pool(name="w0g", bufs=5) as w0g, \
             tc.tile_pool(name="w1g", bufs=5) as w1g, \
             tc.tile_pool(name="wls", bufs=2) as wls, \
             tc.tile_pool(name="acts", bufs=3) as acts, \
             tc.tile_pool(name="bias", bufs=1) as bias, \
             tc.tile_pool(name="ps", bufs=1, space="PSUM") as ps:

            # ---------------- one-time setup ----------------
            # scalar ring: wv0 first (gates the sample-0 dequant), then x/eye
            tmp0 = const.tile([P0, W0C], io_dt, tag="tmp0")
            nc.scalar.dma_start(tmp0[:], wv0[:, :])
            t_std0 = const.tile([P0, W0C], io_dt)
            nc.scalar.activation(t_std0[:], tmp0[:], AF.Exp, scale=0.5)

            t_xT = const.tile([P0, T0 * B], io_dt)
            nc.scalar.dma_start(t_xT[:], xT[:, :])
            t_eye = const.tile([P1, P1], io_dt)
            nc.scalar.dma_start(t_eye[:], eye128[:, :])

            tmp1 = const.tile([P1, W1C], io_dt, tag="tmp1")
            nc.scalar.dma_start(tmp1[:], wv1[:, :])
            t_std1 = const.tile([P1, W1C], io_dt)
            nc.scalar.activation(t_std1[:], tmp1[:], AF.Exp, scale=0.5)

            # scalar ring: small bias/last-layer tensors in parallel
            tmpl = wls.tile([P1, T1 * DO], io_dt, tag="t_wls")
            nc.scalar.dma_start(tmpl[:], wvlT[:, :])
            t_stdl = const.tile([P1, T1 * DO], io_dt)
            nc.scalar.activation(t_stdl[:], tmpl[:], AF.Exp, scale=0.5)
            t_wml = const.tile([P1, T1 * DO], io_dt)
            nc.scalar.dma_start(t_wml[:], wmlT[:, :])
            t_wel = const.tile([P1, SP * T1 * DO], io_dt)
            nc.scalar.dma_start(t_wel[:], welT[:, :])

            # packed hidden biases: [bv0|bm0|be0|bv1|bm1|be1] along free dim
            CB = 2 * C1 + C1 * SP
            t_b01 = bias.tile([P1, 2 * CB], F32, tag="b01")
            nc.scalar.dma_start(t_b01[:], b01[:, :])

            def make_bias_T(off, name):
                vt = t_b01[:, off: off + C1]
                mt = t_b01[:, off + C1: off + 2 * C1]
                et = t_b01[:, off + 2 * C1: off + CB]
                st = bias.tile([P1, C1], F32, tag=name + "s")
                nc.scalar.activation(st[:], vt, AF.Exp, scale=0.5)
                bt = const.tile([P1, C1 * SP], F32, tag=name)
                for c in range(C1):
                    nc.vector.tensor_scalar_mul(
                        bt[:, ts(c, SP)], et[:, ts(c, SP)], st[:, c:c + 1])
                    nc.vector.tensor_scalar_add(
                        bt[:, ts(c, SP)], bt[:, ts(c, SP)], mt[:, c:c + 1])
                return bt

            t_bT0 = make_bias_T(0, "bT0")
            t_bT1 = make_bias_T(CB, "bT1")

            # last-layer bias rows [SP, DO]: bvl/bml pre-replicated on host
            r = bias.tile([SP, DO], io_dt, tag="brow")
            nc.scalar.dma_start(r[:], bvl[:, :])
            sbb = bias.tile([SP, DO], io_dt, tag="brow2")
            nc.scalar.activation(sbb[:], r[:], AF.Exp, scale=0.5)
            mb = bias.tile([SP, DO], io_dt, tag="brow3")
            nc.scalar.dma_start(mb[:], bml[:, :])
            eb = bias.tile([SP, DO], io_dt, tag="bb3")
            nc.scalar.dma_start(eb[:], bel[:, :])
            ba = bias.tile([SP, DO], io_dt, tag="bb4")
            nc.vector.tensor_mul(ba[:], eb[:], sbb[:])
            t_bl = bias.tile([SP, DO], io_dt, tag="ball")
            nc.vector.tensor_add(t_bl[:], ba[:], mb[:])

            t_ind = const.tile([SP, SP * B], io_dt)
            nc.scalar.dma_start(t_ind[:], ind[:, :])

            t_wm0 = const.tile([P0, W0C], io_dt)
            t_wm1 = const.tile([P1, W1C], io_dt)

            t_out = const.tile([B, SP * DO], F32)

            def mm(psum, lhsT, rhs, start, stop, skip=False):
                nc.tensor.matmul(psum, lhsT, rhs, start=start, stop=stop,
                                 skip_group_check=skip)

            # y0T[q, c*64+b] = (x @ wm0)[4q+c, b], precomputed once (bf16)
            def make_y0T():
                y0 = const.tile([P1, C1 * B], io_dt)
                py = ps.tile([P1, C1 * B], F32, tag="py")
                for c in range(C1):
                    for t in range(T0):
                        mm(py[:, ts(c, B)],
                           t_wm0[:, t * D1 + c * P1: t * D1 + (c + 1) * P1],
                           t_xT[:, ts(t, B)],
                           start=(t == 0), stop=(t == T0 - 1))
                nc.scalar.copy(y0[:], py[:])
                return y0

            # ---------------- per-sample weight prep (conveyor) ----------------
            def weight_prep(s, first=False):
                t_e0 = w0g.tile([P0, W0C], io_dt, tag="t_e0")
                nc.sync.dma_start(t_e0[:], we0A[:, s * W0C: (s + 1) * W0C])
                if first:
                    nc.sync.dma_start(t_wm0[:], wm0[:, :])
                t_e1 = w1g.tile([P1, W1C], io_dt, tag="t_e1")
                nc.sync.dma_start(t_e1[:], we1A[:, s * W1C: (s + 1) * W1C])
                if first:
                    nc.sync.dma_start(t_wm1[:], wm1[:, :])

                nc.vector.tensor_mul(t_e0[:], t_e0[:], t_std0[:])
                nc.vector.tensor_mul(t_e1[:], t_e1[:], t_std1[:])
                nc.vector.tensor_add(t_e1[:], t_e1[:], t_wm1[:])
                t_wl = wls.tile([P1, T1 * DO], io_dt, tag="t_wlf")
                nc.vector.tensor_mul(
                    t_wl[:], t_wel[:, s * T1 * DO: (s + 1) * T1 * DO],
                    t_stdl[:])
                nc.vector.tensor_add(t_wl[:], t_wl[:], t_wml[:])
                return t_e0, t_e1, t_wl

            def compute(s, t_e0, t_e1, t_wl, t_y0T, po):
                w0 = t_e0[:]
                w1 = t_e1[:]
                wlf = t_wl[:]

                # layer 0: per-chunk psum tiles so relu(c) overlaps mm(c+1)
                a1T = acts.tile([P1, C1 * B], io_dt, tag="a1T")
                for c in range(C1):
                    pc = ps.tile([P1, B], F32, tag=f"pc{c}")
                    for t in range(T0):
                        mm(pc[:],
                           w0[:, t * D1 + c * P1: t * D1 + (c + 1) * P1],
                           t_xT[:, ts(t, B)],
                           start=(t == 0), stop=False)
                    mm(pc[:], t_eye[:], t_y0T[:, ts(c, B)],
                       start=False, stop=True)
                    nc.scalar.activation(
                        a1T[:, ts(c, B)], pc[:], AF.Relu,
                        bias=t_bT0[:, c * SP + s: c * SP + s + 1])

                # layer 1 (mean already folded into w1)
                a2T = acts.tile([P1, C1 * B], io_dt, tag="a2T")
                for c in range(C1):
                    qc = ps.tile([P1, B], F32, tag=f"pc{c}")
                    for t in range(T1):
                        mm(qc[:],
                           w1[:, t * D2 + c * P1: t * D2 + (c + 1) * P1],
                           a1T[:, ts(t, B)],
                           start=(t == 0), stop=(t == T1 - 1))
                    nc.scalar.activation(
                        a2T[:, ts(c, B)], qc[:], AF.Relu,
                        bias=t_bT1[:, c * SP + s: c * SP + s + 1])

                # output layer: all samples share one [64, SP*DO] psum bank
                for t in range(T1):
                    mm(po[:, ts(s, DO)], a2T[:, ts(t, B)],
                       wlf[:, ts(t, DO)], start=(t == 0), stop=False)
                mm(po[:, ts(s, DO)], t_ind[:, ts(s, B)], t_bl[:],
                   start=False, stop=True)

            po = ps.tile([B, SP * DO], F32, tag="out")
            LOOKAHEAD = 4
            preps = [weight_prep(0, first=True)]
            t_y0T = make_y0T()
            for s in range(1, LOOKAHEAD):
                preps.append(weight_prep(s))
            for s in range(SP):
                compute(s, *preps[s], t_y0T, po)
                if s + LOOKAHEAD < SP:
                    preps.append(weight_prep(s + LOOKAHEAD))
            nc.scalar.copy(t_out[:], po[:])
            nc.sync.dma_start(out[:, :], t_out[:])

    nc.compile()
    return nc


def _get_nc(mode="bf16"):
    if "nc" not in _CACHE:
        _CACHE["nc"] = _build()
    return _CACHE["nc"]


def _prep_in_maps(inputs, mode="bf16"):
    import ml_dtypes
    np_dt = ml_dtypes.bfloat16

    def cvt(a):
        return np.ascontiguousarray(a).astype(np_dt, copy=False)

    x = np.asarray(inputs["inputs"], np.float32)
    we0 = np.asarray(inputs["we0"], np.float32)
    we1 = np.asarray(inputs["we1"], np.float32)
    wel = np.asarray(inputs["wel"], np.float32)
    be0 = np.asarray(inputs["be0"], np.float32).reshape(S, D1)
    be1 = np.asarray(inputs["be1"], np.float32).reshape(S, D2)
    bel = np.asarray(inputs["bel"], np.float32).reshape(S, DO)

    # p-major rows + mod-4 interleaved feature columns:
    #   out[p, (t, c, q)] = M[T*p + t, 4*q + c]
    def pm0(M):  # [784, 512] -> [112, 7*512]
        return M.reshape(P0, T0, P1, C1).transpose(0, 1, 3, 2) \
                .reshape(P0, W0C)

    def pm1(M):  # [512, 512] -> [128, 4*512]
        return M.reshape(P1, T1, P1, C1).transpose(0, 1, 3, 2) \
                .reshape(P1, W1C)

    def pml(M):  # [512, 10] -> [128, 4*10] (row permutation only)
        return M.reshape(P1, T1 * DO)

    xTpm = x.T.reshape(P0, T0, B).reshape(P0, T0 * B)

    def bias_T(b):  # [SP, D] -> [128, C1*SP] with [q, c*SP+s] = b[s, 4q+c]
        return np.ascontiguousarray(
            b.reshape(SP, P1, C1).transpose(1, 2, 0).reshape(P1, C1 * SP))

    def bias_cq(v):  # [D] -> [128, C1] with [q, c] = v[4q+c]
        return np.ascontiguousarray(np.asarray(v, np.float32)
                                    .reshape(P1, C1))

    shared = {
        "xT": cvt(xTpm),
        "wm0": cvt(pm0(np.asarray(inputs["wm0"], np.float32))),
        "wv0": cvt(pm0(np.asarray(inputs["wv0"], np.float32))),
        "wm1": cvt(pm1(np.asarray(inputs["wm1"], np.float32))),
        "wv1": cvt(pm1(np.asarray(inputs["wv1"], np.float32))),
        "wmlT": cvt(pml(np.asarray(inputs["wml"], np.float32))),
        "wvlT": cvt(pml(np.asarray(inputs["wvl"], np.float32))),
        "eye128": cvt(np.eye(P1, dtype=np.float32)),
        "bvl": cvt(np.repeat(np.asarray(inputs["bvl"], np.float32)
                             .reshape(1, DO), SP, axis=0)),
        "bml": cvt(np.repeat(np.asarray(inputs["bml"], np.float32)
                             .reshape(1, DO), SP, axis=0)),
        "ind": cvt(np.repeat(np.eye(SP, dtype=np.float32), B, axis=1)),
    }

    def shard(a, k):
        lo = k * SP
        hi = lo + SP
        if hi <= S:
            return a[lo:hi]
        return np.concatenate([a[lo:S], a[: hi - S]], axis=0)

    def q8(a):
        return np.clip(np.rint(np.ascontiguousarray(a) * 32.0),
                       -127, 127).astype(np.int8)

    in_maps = []
    for k in range(NCORES):
        welk = shard(wel, k)  # [SP, 512, 10]
        b0 = np.concatenate([bias_cq(inputs["bv0"]), bias_cq(inputs["bm0"]),
                             bias_T(shard(be0, k))], axis=1)
        b1 = np.concatenate([bias_cq(inputs["bv1"]), bias_cq(inputs["bm1"]),
                             bias_T(shard(be1, k))], axis=1)
        in_maps.append(dict(
            shared,
            we0A=cvt(np.stack([pm0(m) for m in shard(we0, k)], axis=1)
                     .reshape(P0, SP * W0C)),
            we1A=cvt(np.stack([pm1(m) for m in shard(we1, k)], axis=1)
                     .reshape(P1, SP * W1C)),
            welT=cvt(np.stack([pml(m) for m in welk], axis=1)
                     .reshape(P1, SP * T1 * DO)),
            b01=np.ascontiguousarray(np.concatenate([b0, b1], axis=1)),
            bel=cvt(shard(bel, k)),
        ))
    return in_maps


def _run(inputs, mode="bf16", trace=False):
    nc = _get_nc(mode)
    in_maps = _prep_in_maps(inputs, mode)
    res = run_bass_kernel_spmd(nc, in_maps, core_ids=list(range(NCORES)),
                               trace=trace)
    outs = []
    for k in range(NCORES):
        o = np.asarray(res.results[k]["out"], np.float32)  # [64, 130]
        outs.append(o.reshape(B, SP, DO).transpose(1, 0, 2))
    full = np.concatenate(outs, axis=0)[:S]  # [100, 64, 10]
    return full, res


def kernel(**inputs):
    out, _ = _run(inputs)
    return out
